# revision 26
# baseline (speedup 1.0000x reference)
"""GQA attention block on 8 NeuronCores.

Sharding: tensor-parallel over head groups (4 ways: 8 q heads / 2 kv heads
per core) x data-parallel over batch (2 ways).  Each core computes a partial
y = attn_out_slice @ Wo_slice for its (batch, head-group); the host sums the
4 TP partials per batch element.

v2 design notes (per core, all matmul inputs bf16, PSUM accumulation fp32):
  - Host pre-scales Wq by 1/sqrt(dh) and pre-permutes q heads to
    [0,4,1,5,2,6,3,7] so head h sits at (mb=h%4, partition 64*(h//4)); its
    kv head then lives at the same partition offset in kt, satisfying
    matmul's equal-base-partition rule AND enabling row-tiled S pairs.
  - Phase A: x^T chunks (PE transposes, batched bf16 evictions) + k/v
    projections.  All 4 x^T chunks stay resident so q projections can be
    deferred.
  - Phase B (per 256-col q block): q projection (PE filler that hides under
    exp), then per head-pair (p, p+4): S matmuls use PE row groups 0-1 /
    2-3 (contraction=64) -> issued adjacently they run concurrently; S
    tiles [128,256] are packed 4-per-group into 2 PSUM banks so one ACT
    instruction exps 1024 elems/partition (amortizes ~280cyc/instr
    overhead); PV accumulates [v|1]^T @ ex into a shared pair bank (the
    65th 'ones' row gives the softmax denominator for free).
  - PSUM bank budget (8): pv pairs (1 bank x2) + S groups (2 banks x2) +
    work/aux (1 bank x2).
  - normalization: reciprocal_approx_fast + gpsimd partition broadcast +
    DVE multiply (the plain DVE reciprocal costs 3.3us/instr).
  - NOTE the Tile framework resolves RAW deps in *emission order* only:
    every consumer must be emitted after its producer.  PV's has_written
    gotcha: start=True clears the whole bank, so only the pair's first
    matmul starts the group.
"""

import os
import sys

import numpy as np
import ml_dtypes

for _p in ("/opt/trn_rl_repo",):
    if os.path.isdir(_p) and _p not in sys.path:
        sys.path.insert(0, _p)

from contextlib import ExitStack

import concourse.bass as bass  # noqa: F401  (AP types pulled in transitively)
import concourse.mybir as mybir
import concourse.tile as tile
from concourse import bacc
from concourse.bass_utils import run_bass_kernel_spmd
from concourse.masks import make_identity

P = 128
B, T, D = 2, 2048, 2048
HQ, HKV, DH = 32, 8, 64
GROUP = HQ // HKV            # 4
TP = 4                       # tensor-parallel ways
DP = 2                       # data-parallel ways
NCORES = TP * DP
DQ = D // TP                 # 512 q dims per core (8 heads)
DKV = HKV * DH // TP         # 128 kv dims per core (2 kv heads)
NHQ = HQ // TP               # 8 q heads per core
NKV = HKV // TP              # 2 kv heads per core
NKS = D // P                 # 16 contraction subtiles over D
CH = 512                     # T chunk width in projection phase
NCH = T // CH                # 4
TQ = 256                     # T_q block width in attention
NTQ = T // TQ                # 8
NKI = T // P                 # 16 key tiles
SCALE = 1.0 / 8.0            # 1/sqrt(DH), folded into Wq host-side
F32 = mybir.dt.float32
BF16 = mybir.dt.bfloat16
AF = mybir.ActivationFunctionType
BF = ml_dtypes.bfloat16


def _build():
    nc = bacc.Bacc(None, target_bir_lowering=False, debug=False)

    x_ext = nc.dram_tensor("x", [T, D], BF16, kind="ExternalInput")
    wq_ext = nc.dram_tensor("wq", [D, DQ], BF16, kind="ExternalInput")
    wk_ext = nc.dram_tensor("wk", [D, DKV], BF16, kind="ExternalInput")
    wv_ext = nc.dram_tensor("wv", [D, DKV], BF16, kind="ExternalInput")
    wo_ext = nc.dram_tensor("wo", [DQ, D], BF16, kind="ExternalInput")
    y_ext = nc.dram_tensor("y", [T, D], F32, kind="ExternalOutput")

    x_v = x_ext[:].rearrange("(to p) d -> p to d", p=P)      # [128,16,2048]
    wq_v = wq_ext[:].rearrange("(ko p) m -> p ko m", p=P)    # [128,16,512]
    wk_v = wk_ext[:].rearrange("(ko p) m -> p ko m", p=P)    # [128,16,128]
    wv_v = wv_ext[:].rearrange("(ko p) m -> p ko m", p=P)
    wo_v = wo_ext[:].rearrange("(ko p) n -> p ko n", p=P)    # [128,4,2048]
    y_v = y_ext[:].rearrange("(to p) n -> p to n", p=P)      # [128,16,2048]

    with tile.TileContext(nc) as tc, ExitStack() as ctx:
        const = ctx.enter_context(tc.tile_pool(name="const", bufs=1))
        w_p = ctx.enter_context(tc.tile_pool(name="wp", bufs=1))
        xt_p = ctx.enter_context(tc.tile_pool(name="xtp", bufs=4))
        row_p = ctx.enter_context(tc.tile_pool(name="rows", bufs=2))
        qt_p = ctx.enter_context(tc.tile_pool(name="qt", bufs=1))
        kt_p = ctx.enter_context(tc.tile_pool(name="kt", bufs=1))
        vo_p = ctx.enter_context(tc.tile_pool(name="vo", bufs=1))
        exp_p = ctx.enter_context(tc.tile_pool(name="expp", bufs=4))
        bc_p = ctx.enter_context(tc.tile_pool(name="bcp", bufs=2))
        rc_p = ctx.enter_context(tc.tile_pool(name="rcp", bufs=2))
        ot_p = ctx.enter_context(tc.tile_pool(name="otp", bufs=2))
        y_p = ctx.enter_context(tc.tile_pool(name="yp", bufs=2))

        work_ps = ctx.enter_context(tc.tile_pool(name="work_ps", bufs=2, space="PSUM"))
        s_ps = ctx.enter_context(tc.tile_pool(name="s_ps", bufs=2, space="PSUM"))
        pv_ps = ctx.enter_context(tc.tile_pool(name="pv_ps", bufs=2, space="PSUM"))

        identity = const.tile([P, P], BF16)
        make_identity(nc, identity)

        wq_sb = w_p.tile([P, NKS, DQ], BF16, tag="wq")
        wk_sb = w_p.tile([P, NKS, DKV], BF16, tag="wk")
        wv_sb = w_p.tile([P, NKS, DKV], BF16, tag="wv")
        wo_sb = w_p.tile([P, DQ // P, D], BF16, tag="wo")

        qt_sb = qt_p.tile([P, DQ // P, T], BF16)        # q^T (pre-scaled), [dim, t]
        kt_sb = kt_p.tile([P, T], BF16)                 # k^T, [dim(2 kv heads), t]
        vones = vo_p.tile([P, NKV, NKI, DH + 1], BF16)  # [t%128, kv, t//128, dh|1]
        ones_col = const.tile([P, NKV, NKI], BF16)
        nc.gpsimd.memset(ones_col[:], 1.0)
        nc.vector.tensor_copy(vones[:, :, :, DH], ones_col[:])

        # ---- Phase A: x^T chunks (kept resident) + k/v projections ----
        # k/v projections for chunk c-1 are interleaved into chunk c's
        # transpose stream: real matmuls keep HAM warm (transpose-mode does
        # not count as PE activity) and fill the DVE-eviction stalls.
        def kvproj(c):
            kp = work_ps.tile([P, CH], F32, tag="w")
            for ks in range(NKS):
                nc.tensor.matmul(kp[:], wk_sb[:, ks, :],
                                 xt_chunks[c][:, ks, :],
                                 start=(ks == 0), stop=(ks == NKS - 1))
            nc.vector.tensor_copy(kt_sb[:, c * CH:(c + 1) * CH], kp[:])
            vp = work_ps.tile([P, CH], F32, tag="w")
            for ks in range(NKS):
                nc.tensor.matmul(vp[:], wv_sb[:, ks, :],
                                 xt_chunks[c][:, ks, :],
                                 start=(ks == 0), stop=(ks == NKS - 1))
            vt_sb = row_p.tile([P, CH], BF16, tag="vt")
            nc.vector.tensor_copy(vt_sb[:], vp[:])
            vtp = work_ps.tile([P, CH // P, P], BF16, tag="w")
            for r in range(CH // P):
                nc.tensor.transpose(vtp[:, r, :], vt_sb[:, r * P:(r + 1) * P],
                                    identity)
            for j in range(NKV):
                nc.vector.tensor_copy(
                    vones[:, j, c * (CH // P):(c + 1) * (CH // P), 0:DH],
                    vtp[:, :, j * DH:(j + 1) * DH])

        xt_chunks = []
        for c in range(NCH):
            xt_ch = xt_p.tile([P, NKS, CH], BF16, tag="xt")  # x^T[:, c*CH:+CH]
            xt_chunks.append(xt_ch)
            for r in range(CH // P):
                xrow = row_p.tile([P, D], BF16, tag="rows")
                nc.sync.dma_start(xrow[:], x_v[:, c * (CH // P) + r, :])
                for g in range(2):
                    tp = work_ps.tile([P, 8, P], BF16, tag="w")
                    for t8 in range(8):
                        dsb = g * 8 + t8
                        nc.tensor.transpose(
                            tp[:, t8, :], xrow[:, dsb * P:(dsb + 1) * P], identity)
                    nc.vector.tensor_copy(
                        xt_ch[:, g * 8:(g + 1) * 8, r * P:(r + 1) * P], tp[:])
                if c == 0 and r == 0:
                    # weights go out after the first x rows so PE transposes
                    # start as early as possible
                    nc.sync.dma_start(wq_sb[:], wq_v)
                    nc.sync.dma_start(wk_sb[:], wk_v)
                    nc.sync.dma_start(wv_sb[:], wv_v)
                    nc.sync.dma_start(wo_sb[:], wo_v)
                if c > 0 and r == 1:
                    kvproj(c - 1)
        kvproj(NCH - 1)

        # ---- Phase B: per T_q block: q proj (exp-hiding PE filler),
        #      attention head-pairs, Wo ----
        def qproj(tb, mb):
            xt_ch = xt_chunks[tb // 2]
            xcols = slice((tb % 2) * TQ, (tb % 2 + 1) * TQ)
            qp = work_ps.tile([P, TQ], F32, tag="w")
            for ks in range(NKS):
                nc.tensor.matmul(
                    qp[:], wq_sb[:, ks, mb * P:(mb + 1) * P],
                    xt_ch[:, ks, xcols],
                    start=(ks == 0), stop=(ks == NKS - 1))
            nc.vector.tensor_copy(qt_sb[:, mb, tb * TQ:(tb + 1) * TQ], qp[:])

        for mb in range(DQ // P):
            qproj(0, mb)
        prev = None  # (outt, tb, y_sbs) of the previous block
        for tb in range(NTQ):
            cols = slice(tb * TQ, (tb + 1) * TQ)
            outt = ot_p.tile([P, DQ // P, TQ], BF16, tag="ot")
            for p in range(TP):  # head pair (p, p+4); kv heads (0, 1)
                # both heads share one bank; start=True clears the whole
                # bank's has_written bits so ONLY the pair's first matmul
                # starts -- head B's first then overwrites-on-clear.
                pv = pv_ps.tile([P, NKV, TQ], F32, tag="pv")
                # filler for this pair, fine-grained per kg step so the PE
                # always has non-exp-dependent work while ACT streams exps:
                #   2 qproj(tb+1, mb=p) matmuls + 1 Wo(tb-1) matmul per kg
                do_q = tb + 1 < NTQ
                if do_q:
                    xt_nx = xt_chunks[(tb + 1) // 2]
                    xc_nx = slice(((tb + 1) % 2) * TQ, ((tb + 1) % 2 + 1) * TQ)
                    qp = work_ps.tile([P, TQ], F32, tag="w")
                yp = None
                for kg in range(NKI // 2):
                    sg = s_ps.tile([P, 4, TQ], F32, tag="s")
                    ex = exp_p.tile([P, 4, TQ], BF16, tag="exp")
                    for kl in range(2):
                        ki = kg * 2 + kl
                        kc = slice(ki * P, (ki + 1) * P)
                        # bank0 holds A(ki0),A(ki1); bank1 B(ki0),B(ki1)
                        # -> the concurrent (A,B) pair hits 2 banks
                        nc.tensor.matmul(
                            sg[:, kl, :], kt_sb[0:DH, kc],
                            qt_sb[0:DH, p, cols], start=True, stop=True)
                        nc.tensor.matmul(
                            sg[:, 2 + kl, :], kt_sb[DH:2 * DH, kc],
                            qt_sb[DH:2 * DH, p, cols], start=True, stop=True)
                    nc.scalar.activation(ex[:], sg[:], AF.Exp)
                    for kl in range(2):
                        ki = kg * 2 + kl
                        nc.tensor.matmul(
                            pv[0:DH + 1, 0, :], vones[:, 0, ki, :],
                            ex[:, kl, :],
                            start=(ki == 0), stop=(ki == NKI - 1),
                            skip_group_check=True)
                        nc.tensor.matmul(
                            pv[0:DH + 1, 1, :], vones[:, 1, ki, :],
                            ex[:, 2 + kl, :],
                            start=False, stop=(ki == NKI - 1),
                            skip_group_check=True)
                    # --- filler ---
                    if do_q:
                        for ks in (2 * kg, 2 * kg + 1):
                            nc.tensor.matmul(
                                qp[:], wq_sb[:, ks, p * P:(p + 1) * P],
                                xt_nx[:, ks, xc_nx],
                                start=(ks == 0), stop=(ks == NKS - 1))
                    if prev is not None:
                        po, pt, py = prev
                        g = 2 * p + kg // 4        # Wo group (mi, nb)
                        mi, nb = divmod(g, 4)
                        ks = kg % 4
                        if ks == 0:
                            yp = work_ps.tile([P, 512], F32, tag="w")
                        nc.tensor.matmul(
                            yp[:], po[:, ks, mi * P:(mi + 1) * P],
                            wo_sb[:, ks, nb * 512:(nb + 1) * 512],
                            start=(ks == 0), stop=(ks == DQ // P - 1))
                        if ks == 3:
                            nc.vector.tensor_copy(
                                py[mi][:, nb * 512:(nb + 1) * 512], yp[:])
                            if nb == 3:
                                mt = pt * (TQ // P) + mi
                                nc.sync.dma_start(y_v[:, mt, :], py[mi][:])
                if do_q:
                    nc.vector.tensor_copy(
                        qt_sb[:, p, (tb + 1) * TQ:(tb + 2) * TQ], qp[:])
                # denominator -> SBUF first: reciprocal_approx_fast's custom
                # DVE microcode mis-reads PSUM operands (verified on HW)
                den = rc_p.tile([1, NKV, TQ], F32, tag="den")
                nc.vector.tensor_copy(den[:], pv[DH:DH + 1, :, :])
                rc = rc_p.tile([1, NKV, TQ], F32, tag="rc")
                nc.vector.reciprocal_approx_fast(rc[:], den[:])
                bcst = bc_p.tile([DH, NKV, TQ], F32, tag="bc")
                nc.gpsimd.partition_broadcast(bcst[:], rc[:], channels=DH)
                nc.vector.tensor_mul(
                    outt[0:DH, p, :], pv[0:DH, 0, :], bcst[:, 0, :])
                nc.vector.tensor_mul(
                    outt[DH:2 * DH, p, :], pv[0:DH, 1, :], bcst[:, 1, :])
            y_sbs = [y_p.tile([P, D], F32, tag="y", name=f"ysb{tb}_{i}")
                     for i in range(2)]
            prev = (outt, tb, y_sbs)
        # Wo for the final block (no next block to hide it under)
        po, pt, py = prev
        for g in range(8):
            mi, nb = divmod(g, 4)
            yp = work_ps.tile([P, 512], F32, tag="w")
            for ks in range(DQ // P):
                nc.tensor.matmul(
                    yp[:], po[:, ks, mi * P:(mi + 1) * P],
                    wo_sb[:, ks, nb * 512:(nb + 1) * 512],
                    start=(ks == 0), stop=(ks == DQ // P - 1))
            nc.vector.tensor_copy(py[mi][:, nb * 512:(nb + 1) * 512], yp[:])
            if nb == 3:
                mt = pt * (TQ // P) + mi
                nc.sync.dma_start(y_v[:, mt, :], py[mi][:])

    nc.compile()
    return nc


_NC_CACHE = {}


def _get_nc():
    if "nc" not in _NC_CACHE:
        _NC_CACHE["nc"] = _build()
    return _NC_CACHE["nc"]


# q-head interleave: head h of the core -> (mb h%4, partition 64*(h//4))
_PERM = np.concatenate(
    [np.r_[b * DH:(b + 1) * DH, (b + 4) * DH:(b + 5) * DH] for b in range(4)])


def make_in_maps(x, Wq, Wk, Wv, Wo):
    x = np.asarray(x, dtype=np.float32)
    Wq = np.asarray(Wq, dtype=np.float32) * SCALE
    Wk = np.asarray(Wk, dtype=np.float32)
    Wv = np.asarray(Wv, dtype=np.float32)
    Wo = np.asarray(Wo, dtype=np.float32)
    in_maps = []
    for c in range(NCORES):
        b, g = divmod(c, TP)
        in_maps.append({
            "x": np.ascontiguousarray(x[b]).astype(BF),
            "wq": np.ascontiguousarray(
                Wq[:, g * DQ:(g + 1) * DQ][:, _PERM]).astype(BF),
            "wk": np.ascontiguousarray(Wk[:, g * DKV:(g + 1) * DKV]).astype(BF),
            "wv": np.ascontiguousarray(Wv[:, g * DKV:(g + 1) * DKV]).astype(BF),
            "wo": np.ascontiguousarray(
                Wo[g * DQ:(g + 1) * DQ, :][_PERM, :]).astype(BF),
        })
    return in_maps


def kernel(x, Wq, Wk, Wv, Wo):
    nc = _get_nc()
    in_maps = make_in_maps(x, Wq, Wk, Wv, Wo)
    res = run_bass_kernel_spmd(nc, in_maps, list(range(NCORES)))
    y = np.zeros((B, T, D), dtype=np.float32)
    for c in range(NCORES):
        b = c // TP
        y[b] += res.results[c]["y"]
    return y


# revision 27
# speedup vs baseline: 1.0368x; 1.0368x over previous
"""GQA attention block on 8 NeuronCores.

Sharding: tensor-parallel over head groups (4 ways: 8 q heads / 2 kv heads
per core) x data-parallel over batch (2 ways).  Each core computes a partial
y = attn_out_slice @ Wo_slice for its (batch, head-group); the host sums the
4 TP partials per batch element.

v2 design notes (per core, all matmul inputs bf16, PSUM accumulation fp32):
  - Host pre-scales Wq by 1/sqrt(dh) and pre-permutes q heads to
    [0,4,1,5,2,6,3,7] so head h sits at (mb=h%4, partition 64*(h//4)); its
    kv head then lives at the same partition offset in kt, satisfying
    matmul's equal-base-partition rule AND enabling row-tiled S pairs.
  - Phase A: x^T chunks (PE transposes, batched bf16 evictions) + k/v
    projections.  All 4 x^T chunks stay resident so q projections can be
    deferred.
  - Phase B (per 256-col q block): q projection (PE filler that hides under
    exp), then per head-pair (p, p+4): S matmuls use PE row groups 0-1 /
    2-3 (contraction=64) -> issued adjacently they run concurrently; S
    tiles [128,256] are packed 4-per-group into 2 PSUM banks so one ACT
    instruction exps 1024 elems/partition (amortizes ~280cyc/instr
    overhead); PV accumulates [v|1]^T @ ex into a shared pair bank (the
    65th 'ones' row gives the softmax denominator for free).
  - PSUM bank budget (8): pv pairs (1 bank x2) + S groups (2 banks x2) +
    work/aux (1 bank x2).
  - normalization: reciprocal_approx_fast + gpsimd partition broadcast +
    DVE multiply (the plain DVE reciprocal costs 3.3us/instr).
  - NOTE the Tile framework resolves RAW deps in *emission order* only:
    every consumer must be emitted after its producer.  PV's has_written
    gotcha: start=True clears the whole bank, so only the pair's first
    matmul starts the group.
"""

import os
import sys

import numpy as np
import ml_dtypes

for _p in ("/opt/trn_rl_repo",):
    if os.path.isdir(_p) and _p not in sys.path:
        sys.path.insert(0, _p)

from contextlib import ExitStack

import concourse.bass as bass  # noqa: F401  (AP types pulled in transitively)
import concourse.mybir as mybir
import concourse.tile as tile
from concourse import bacc
from concourse.bass_utils import run_bass_kernel_spmd
from concourse.masks import make_identity

P = 128
B, T, D = 2, 2048, 2048
HQ, HKV, DH = 32, 8, 64
GROUP = HQ // HKV            # 4
TP = 4                       # tensor-parallel ways
DP = 2                       # data-parallel ways
NCORES = TP * DP
DQ = D // TP                 # 512 q dims per core (8 heads)
DKV = HKV * DH // TP         # 128 kv dims per core (2 kv heads)
NHQ = HQ // TP               # 8 q heads per core
NKV = HKV // TP              # 2 kv heads per core
NKS = D // P                 # 16 contraction subtiles over D
CH = 512                     # T chunk width in projection phase
NCH = T // CH                # 4
TQ = 256                     # T_q block width in attention
NTQ = T // TQ                # 8
NKI = T // P                 # 16 key tiles
SCALE = 1.0 / 8.0            # 1/sqrt(DH), folded into Wq host-side
F32 = mybir.dt.float32
BF16 = mybir.dt.bfloat16
AF = mybir.ActivationFunctionType
BF = ml_dtypes.bfloat16


def _build():
    nc = bacc.Bacc(None, target_bir_lowering=False, debug=False)

    x_ext = nc.dram_tensor("x", [T, D], BF16, kind="ExternalInput")
    wq_ext = nc.dram_tensor("wq", [D, DQ], BF16, kind="ExternalInput")
    wk_ext = nc.dram_tensor("wk", [D, DKV], BF16, kind="ExternalInput")
    wv_ext = nc.dram_tensor("wv", [D, DKV], BF16, kind="ExternalInput")
    wo_ext = nc.dram_tensor("wo", [DQ, D], BF16, kind="ExternalInput")
    y_ext = nc.dram_tensor("y", [T, D], F32, kind="ExternalOutput")

    x_v = x_ext[:].rearrange("(to p) d -> p to d", p=P)      # [128,16,2048]
    wq_v = wq_ext[:].rearrange("(ko p) m -> p ko m", p=P)    # [128,16,512]
    wk_v = wk_ext[:].rearrange("(ko p) m -> p ko m", p=P)    # [128,16,128]
    wv_v = wv_ext[:].rearrange("(ko p) m -> p ko m", p=P)
    wo_v = wo_ext[:].rearrange("(ko p) n -> p ko n", p=P)    # [128,4,2048]
    y_v = y_ext[:].rearrange("(to p) n -> p to n", p=P)      # [128,16,2048]

    with tile.TileContext(nc) as tc, ExitStack() as ctx:
        const = ctx.enter_context(tc.tile_pool(name="const", bufs=1))
        w_p = ctx.enter_context(tc.tile_pool(name="wp", bufs=1))
        xt_p = ctx.enter_context(tc.tile_pool(name="xtp", bufs=4))
        row_p = ctx.enter_context(tc.tile_pool(name="rows", bufs=2))
        qt_p = ctx.enter_context(tc.tile_pool(name="qt", bufs=1))
        kt_p = ctx.enter_context(tc.tile_pool(name="kt", bufs=1))
        vo_p = ctx.enter_context(tc.tile_pool(name="vo", bufs=1))
        exp_p = ctx.enter_context(tc.tile_pool(name="expp", bufs=4))
        bc_p = ctx.enter_context(tc.tile_pool(name="bcp", bufs=2))
        rc_p = ctx.enter_context(tc.tile_pool(name="rcp", bufs=2))
        ot_p = ctx.enter_context(tc.tile_pool(name="otp", bufs=2))
        y_p = ctx.enter_context(tc.tile_pool(name="yp", bufs=2))

        work_ps = ctx.enter_context(tc.tile_pool(name="work_ps", bufs=2, space="PSUM"))
        s_ps = ctx.enter_context(tc.tile_pool(name="s_ps", bufs=2, space="PSUM"))
        pv_ps = ctx.enter_context(tc.tile_pool(name="pv_ps", bufs=2, space="PSUM"))

        identity = const.tile([P, P], BF16)
        make_identity(nc, identity)

        wq_sb = w_p.tile([P, NKS, DQ], BF16, tag="wq")
        wk_sb = w_p.tile([P, NKS, DKV], BF16, tag="wk")
        wv_sb = w_p.tile([P, NKS, DKV], BF16, tag="wv")
        wo_sb = w_p.tile([P, DQ // P, D], BF16, tag="wo")

        qt_sb = qt_p.tile([P, DQ // P, T], BF16)        # q^T (pre-scaled), [dim, t]
        kt_sb = kt_p.tile([P, T], BF16)                 # k^T, [dim(2 kv heads), t]
        vones = vo_p.tile([P, NKV, NKI, DH + 1], BF16)  # [t%128, kv, t//128, dh|1]
        ones_col = const.tile([P, NKV, NKI], BF16)
        nc.gpsimd.memset(ones_col[:], 1.0)
        nc.vector.tensor_copy(vones[:, :, :, DH], ones_col[:])

        # ---- Phase A: x^T chunks (kept resident) + k/v projections ----
        # k/v projections for chunk c-1 are interleaved into chunk c's
        # transpose stream: real matmuls keep HAM warm (transpose-mode does
        # not count as PE activity) and fill the DVE-eviction stalls.
        def kvproj(c):
            kp = work_ps.tile([P, CH], F32, tag="w")
            for ks in range(NKS):
                nc.tensor.matmul(kp[:], wk_sb[:, ks, :],
                                 xt_chunks[c][:, ks, :],
                                 start=(ks == 0), stop=(ks == NKS - 1))
            nc.vector.tensor_copy(kt_sb[:, c * CH:(c + 1) * CH], kp[:])
            vp = work_ps.tile([P, CH], F32, tag="w")
            for ks in range(NKS):
                nc.tensor.matmul(vp[:], wv_sb[:, ks, :],
                                 xt_chunks[c][:, ks, :],
                                 start=(ks == 0), stop=(ks == NKS - 1))
            vt_sb = row_p.tile([P, CH], BF16, tag="vt")
            nc.vector.tensor_copy(vt_sb[:], vp[:])
            vtp = work_ps.tile([P, CH // P, P], BF16, tag="w")
            for r in range(CH // P):
                nc.tensor.transpose(vtp[:, r, :], vt_sb[:, r * P:(r + 1) * P],
                                    identity)
            for j in range(NKV):
                nc.vector.tensor_copy(
                    vones[:, j, c * (CH // P):(c + 1) * (CH // P), 0:DH],
                    vtp[:, :, j * DH:(j + 1) * DH])

        xt_chunks = []
        for c in range(NCH):
            xt_ch = xt_p.tile([P, NKS, CH], BF16, tag="xt")  # x^T[:, c*CH:+CH]
            xt_chunks.append(xt_ch)
            for r in range(CH // P):
                xrow = row_p.tile([P, D], BF16, tag="rows")
                nc.sync.dma_start(xrow[:], x_v[:, c * (CH // P) + r, :])
                # transposes batch through the (phase-A-idle) S pool so they
                # never contend with the k/v projections' work slots
                tp = s_ps.tile([P, NKS, P], BF16, tag="s")
                for dsb in range(NKS):
                    nc.tensor.transpose(
                        tp[:, dsb, :], xrow[:, dsb * P:(dsb + 1) * P], identity)
                nc.vector.tensor_copy(
                    xt_ch[:, :, r * P:(r + 1) * P], tp[:])
                if c == 0 and r == 0:
                    # weights go out after the first x rows so PE transposes
                    # start as early as possible
                    nc.sync.dma_start(wq_sb[:], wq_v)
                    nc.sync.dma_start(wk_sb[:], wk_v)
                    nc.sync.dma_start(wv_sb[:], wv_v)
                    nc.sync.dma_start(wo_sb[:], wo_v)
                if c > 0 and r == 1:
                    kvproj(c - 1)
        kvproj(NCH - 1)

        # ---- Phase B: per T_q block: q proj (exp-hiding PE filler),
        #      attention head-pairs, Wo ----
        def qproj(tb, mb):
            xt_ch = xt_chunks[tb // 2]
            xcols = slice((tb % 2) * TQ, (tb % 2 + 1) * TQ)
            qp = work_ps.tile([P, TQ], F32, tag="w")
            for ks in range(NKS):
                nc.tensor.matmul(
                    qp[:], wq_sb[:, ks, mb * P:(mb + 1) * P],
                    xt_ch[:, ks, xcols],
                    start=(ks == 0), stop=(ks == NKS - 1))
            nc.vector.tensor_copy(qt_sb[:, mb, tb * TQ:(tb + 1) * TQ], qp[:])

        for mb in range(DQ // P):
            qproj(0, mb)
        prev = None  # (outt, tb, y_sbs) of the previous block
        for tb in range(NTQ):
            cols = slice(tb * TQ, (tb + 1) * TQ)
            outt = ot_p.tile([P, DQ // P, TQ], BF16, tag="ot")
            for p in range(TP):  # head pair (p, p+4); kv heads (0, 1)
                # both heads share one bank; start=True clears the whole
                # bank's has_written bits so ONLY the pair's first matmul
                # starts -- head B's first then overwrites-on-clear.
                pv = pv_ps.tile([P, NKV, TQ], F32, tag="pv")
                # filler for this pair, fine-grained per kg step so the PE
                # always has non-exp-dependent work while ACT streams exps:
                #   2 qproj(tb+1, mb=p) matmuls + 1 Wo(tb-1) matmul per kg
                do_q = tb + 1 < NTQ
                if do_q:
                    xt_nx = xt_chunks[(tb + 1) // 2]
                    xc_nx = slice(((tb + 1) % 2) * TQ, ((tb + 1) % 2 + 1) * TQ)
                    qp = work_ps.tile([P, TQ], F32, tag="w")
                yp = None
                for kg in range(NKI // 2):
                    sg = s_ps.tile([P, 4, TQ], F32, tag="s")
                    ex = exp_p.tile([P, 4, TQ], BF16, tag="exp")
                    for kl in range(2):
                        ki = kg * 2 + kl
                        kc = slice(ki * P, (ki + 1) * P)
                        # bank0 holds A(ki0),A(ki1); bank1 B(ki0),B(ki1)
                        # -> the concurrent (A,B) pair hits 2 banks
                        nc.tensor.matmul(
                            sg[:, kl, :], kt_sb[0:DH, kc],
                            qt_sb[0:DH, p, cols], start=True, stop=True)
                        nc.tensor.matmul(
                            sg[:, 2 + kl, :], kt_sb[DH:2 * DH, kc],
                            qt_sb[DH:2 * DH, p, cols], start=True, stop=True)
                    nc.scalar.activation(ex[:], sg[:], AF.Exp)
                    for kl in range(2):
                        ki = kg * 2 + kl
                        nc.tensor.matmul(
                            pv[0:DH + 1, 0, :], vones[:, 0, ki, :],
                            ex[:, kl, :],
                            start=(ki == 0), stop=(ki == NKI - 1),
                            skip_group_check=True)
                        nc.tensor.matmul(
                            pv[0:DH + 1, 1, :], vones[:, 1, ki, :],
                            ex[:, 2 + kl, :],
                            start=False, stop=(ki == NKI - 1),
                            skip_group_check=True)
                    # --- filler ---
                    if do_q:
                        for ks in (2 * kg, 2 * kg + 1):
                            nc.tensor.matmul(
                                qp[:], wq_sb[:, ks, p * P:(p + 1) * P],
                                xt_nx[:, ks, xc_nx],
                                start=(ks == 0), stop=(ks == NKS - 1))
                    if prev is not None:
                        po, pt, py = prev
                        g = 2 * p + kg // 4        # Wo group (mi, nb)
                        mi, nb = divmod(g, 4)
                        ks = kg % 4
                        if ks == 0:
                            yp = work_ps.tile([P, 512], F32, tag="w")
                        nc.tensor.matmul(
                            yp[:], po[:, ks, mi * P:(mi + 1) * P],
                            wo_sb[:, ks, nb * 512:(nb + 1) * 512],
                            start=(ks == 0), stop=(ks == DQ // P - 1))
                        if ks == 3:
                            nc.vector.tensor_copy(
                                py[mi][:, nb * 512:(nb + 1) * 512], yp[:])
                            if nb == 3:
                                mt = pt * (TQ // P) + mi
                                nc.sync.dma_start(y_v[:, mt, :], py[mi][:])
                if do_q:
                    nc.vector.tensor_copy(
                        qt_sb[:, p, (tb + 1) * TQ:(tb + 2) * TQ], qp[:])
                # denominator -> SBUF first: reciprocal_approx_fast's custom
                # DVE microcode mis-reads PSUM operands (verified on HW)
                den = rc_p.tile([1, NKV, TQ], F32, tag="den")
                nc.vector.tensor_copy(den[:], pv[DH:DH + 1, :, :])
                rc = rc_p.tile([1, NKV, TQ], F32, tag="rc")
                nc.vector.reciprocal_approx_fast(rc[:], den[:])
                bcst = bc_p.tile([DH, NKV, TQ], F32, tag="bc")
                nc.gpsimd.partition_broadcast(bcst[:], rc[:], channels=DH)
                nc.vector.tensor_mul(
                    outt[0:DH, p, :], pv[0:DH, 0, :], bcst[:, 0, :])
                nc.vector.tensor_mul(
                    outt[DH:2 * DH, p, :], pv[0:DH, 1, :], bcst[:, 1, :])
            y_sbs = [y_p.tile([P, D], F32, tag="y", name=f"ysb{tb}_{i}")
                     for i in range(2)]
            prev = (outt, tb, y_sbs)
        # Wo for the final block (no next block to hide it under)
        po, pt, py = prev
        for g in range(8):
            mi, nb = divmod(g, 4)
            yp = work_ps.tile([P, 512], F32, tag="w")
            for ks in range(DQ // P):
                nc.tensor.matmul(
                    yp[:], po[:, ks, mi * P:(mi + 1) * P],
                    wo_sb[:, ks, nb * 512:(nb + 1) * 512],
                    start=(ks == 0), stop=(ks == DQ // P - 1))
            nc.vector.tensor_copy(py[mi][:, nb * 512:(nb + 1) * 512], yp[:])
            if nb == 3:
                mt = pt * (TQ // P) + mi
                nc.sync.dma_start(y_v[:, mt, :], py[mi][:])

    nc.compile()
    return nc


_NC_CACHE = {}


def _get_nc():
    if "nc" not in _NC_CACHE:
        _NC_CACHE["nc"] = _build()
    return _NC_CACHE["nc"]


# q-head interleave: head h of the core -> (mb h%4, partition 64*(h//4))
_PERM = np.concatenate(
    [np.r_[b * DH:(b + 1) * DH, (b + 4) * DH:(b + 5) * DH] for b in range(4)])


def make_in_maps(x, Wq, Wk, Wv, Wo):
    x = np.asarray(x, dtype=np.float32)
    Wq = np.asarray(Wq, dtype=np.float32) * SCALE
    Wk = np.asarray(Wk, dtype=np.float32)
    Wv = np.asarray(Wv, dtype=np.float32)
    Wo = np.asarray(Wo, dtype=np.float32)
    in_maps = []
    for c in range(NCORES):
        b, g = divmod(c, TP)
        in_maps.append({
            "x": np.ascontiguousarray(x[b]).astype(BF),
            "wq": np.ascontiguousarray(
                Wq[:, g * DQ:(g + 1) * DQ][:, _PERM]).astype(BF),
            "wk": np.ascontiguousarray(Wk[:, g * DKV:(g + 1) * DKV]).astype(BF),
            "wv": np.ascontiguousarray(Wv[:, g * DKV:(g + 1) * DKV]).astype(BF),
            "wo": np.ascontiguousarray(
                Wo[g * DQ:(g + 1) * DQ, :][_PERM, :]).astype(BF),
        })
    return in_maps


def kernel(x, Wq, Wk, Wv, Wo):
    nc = _get_nc()
    in_maps = make_in_maps(x, Wq, Wk, Wv, Wo)
    res = run_bass_kernel_spmd(nc, in_maps, list(range(NCORES)))
    y = np.zeros((B, T, D), dtype=np.float32)
    for c in range(NCORES):
        b = c // TP
        y[b] += res.results[c]["y"]
    return y


# revision 28
# speedup vs baseline: 1.0671x; 1.0292x over previous
"""GQA attention block on 8 NeuronCores.

Sharding: tensor-parallel over head groups (4 ways: 8 q heads / 2 kv heads
per core) x data-parallel over batch (2 ways).  Each core computes a partial
y = attn_out_slice @ Wo_slice for its (batch, head-group); the host sums the
4 TP partials per batch element.

v2 design notes (per core, all matmul inputs bf16, PSUM accumulation fp32):
  - Host pre-scales Wq by 1/sqrt(dh) and pre-permutes q heads to
    [0,4,1,5,2,6,3,7] so head h sits at (mb=h%4, partition 64*(h//4)); its
    kv head then lives at the same partition offset in kt, satisfying
    matmul's equal-base-partition rule AND enabling row-tiled S pairs.
  - Phase A: x^T chunks (PE transposes, batched bf16 evictions) + k/v
    projections.  All 4 x^T chunks stay resident so q projections can be
    deferred.
  - Phase B (per 256-col q block): q projection (PE filler that hides under
    exp), then per head-pair (p, p+4): S matmuls use PE row groups 0-1 /
    2-3 (contraction=64) -> issued adjacently they run concurrently; S
    tiles [128,256] are packed 4-per-group into 2 PSUM banks so one ACT
    instruction exps 1024 elems/partition (amortizes ~280cyc/instr
    overhead); PV accumulates [v|1]^T @ ex into a shared pair bank (the
    65th 'ones' row gives the softmax denominator for free).
  - PSUM bank budget (8): pv pairs (1 bank x2) + S groups (2 banks x2) +
    work/aux (1 bank x2).
  - normalization: reciprocal_approx_fast + gpsimd partition broadcast +
    DVE multiply (the plain DVE reciprocal costs 3.3us/instr).
  - NOTE the Tile framework resolves RAW deps in *emission order* only:
    every consumer must be emitted after its producer.  PV's has_written
    gotcha: start=True clears the whole bank, so only the pair's first
    matmul starts the group.
"""

import os
import sys

import numpy as np
import ml_dtypes

for _p in ("/opt/trn_rl_repo",):
    if os.path.isdir(_p) and _p not in sys.path:
        sys.path.insert(0, _p)

from contextlib import ExitStack

import concourse.bass as bass  # noqa: F401  (AP types pulled in transitively)
import concourse.mybir as mybir
import concourse.tile as tile
from concourse import bacc
from concourse.bass_utils import run_bass_kernel_spmd
from concourse.masks import make_identity

P = 128
B, T, D = 2, 2048, 2048
HQ, HKV, DH = 32, 8, 64
GROUP = HQ // HKV            # 4
TP = 4                       # tensor-parallel ways
DP = 2                       # data-parallel ways
NCORES = TP * DP
DQ = D // TP                 # 512 q dims per core (8 heads)
DKV = HKV * DH // TP         # 128 kv dims per core (2 kv heads)
NHQ = HQ // TP               # 8 q heads per core
NKV = HKV // TP              # 2 kv heads per core
NKS = D // P                 # 16 contraction subtiles over D
CH = 512                     # T chunk width in projection phase
NCH = T // CH                # 4
TQ = 256                     # T_q block width in attention
NTQ = T // TQ                # 8
NKI = T // P                 # 16 key tiles
SCALE = 1.0 / 8.0            # 1/sqrt(DH), folded into Wq host-side
F32 = mybir.dt.float32
BF16 = mybir.dt.bfloat16
AF = mybir.ActivationFunctionType
BF = ml_dtypes.bfloat16


def _build():
    nc = bacc.Bacc(None, target_bir_lowering=False, debug=False)

    x_ext = nc.dram_tensor("x", [T, D], BF16, kind="ExternalInput")
    wq_ext = nc.dram_tensor("wq", [D, DQ], BF16, kind="ExternalInput")
    wk_ext = nc.dram_tensor("wk", [D, DKV], BF16, kind="ExternalInput")
    wv_ext = nc.dram_tensor("wv", [D, DKV], BF16, kind="ExternalInput")
    wo_ext = nc.dram_tensor("wo", [DQ, D], BF16, kind="ExternalInput")
    y_ext = nc.dram_tensor("y", [T, D], F32, kind="ExternalOutput")

    x_v = x_ext[:].rearrange("(to p) d -> p to d", p=P)      # [128,16,2048]
    wq_v = wq_ext[:].rearrange("(ko p) m -> p ko m", p=P)    # [128,16,512]
    wk_v = wk_ext[:].rearrange("(ko p) m -> p ko m", p=P)    # [128,16,128]
    wv_v = wv_ext[:].rearrange("(ko p) m -> p ko m", p=P)
    wo_v = wo_ext[:].rearrange("(ko p) n -> p ko n", p=P)    # [128,4,2048]
    y_v = y_ext[:].rearrange("(to p) n -> p to n", p=P)      # [128,16,2048]

    with tile.TileContext(nc) as tc, ExitStack() as ctx:
        const = ctx.enter_context(tc.tile_pool(name="const", bufs=1))
        w_p = ctx.enter_context(tc.tile_pool(name="wp", bufs=1))
        xt_p = ctx.enter_context(tc.tile_pool(name="xtp", bufs=4))
        row_p = ctx.enter_context(tc.tile_pool(name="rows", bufs=2))
        qt_p = ctx.enter_context(tc.tile_pool(name="qt", bufs=1))
        kt_p = ctx.enter_context(tc.tile_pool(name="kt", bufs=1))
        vo_p = ctx.enter_context(tc.tile_pool(name="vo", bufs=1))
        exp_p = ctx.enter_context(tc.tile_pool(name="expp", bufs=4))
        bc_p = ctx.enter_context(tc.tile_pool(name="bcp", bufs=2))
        rc_p = ctx.enter_context(tc.tile_pool(name="rcp", bufs=2))
        ot_p = ctx.enter_context(tc.tile_pool(name="otp", bufs=2))
        y_p = ctx.enter_context(tc.tile_pool(name="yp", bufs=2))

        work_ps = ctx.enter_context(tc.tile_pool(name="work_ps", bufs=2, space="PSUM"))
        s_ps = ctx.enter_context(tc.tile_pool(name="s_ps", bufs=2, space="PSUM"))
        pv_ps = ctx.enter_context(tc.tile_pool(name="pv_ps", bufs=2, space="PSUM"))

        identity = const.tile([P, P], BF16)
        make_identity(nc, identity)

        wq_sb = w_p.tile([P, NKS, DQ], BF16, tag="wq")
        wk_sb = w_p.tile([P, NKS, DKV], BF16, tag="wk")
        wv_sb = w_p.tile([P, NKS, DKV], BF16, tag="wv")
        wo_sb = w_p.tile([P, DQ // P, D], BF16, tag="wo")

        qt_sb = qt_p.tile([P, DQ // P, T], BF16)        # q^T (pre-scaled), [dim, t]
        kt_sb = kt_p.tile([P, T], BF16)                 # k^T, [dim(2 kv heads), t]
        vones = vo_p.tile([P, NKV, NKI, DH + 1], BF16)  # [t%128, kv, t//128, dh|1]
        ones_col = const.tile([P, NKV, NKI], BF16)
        nc.gpsimd.memset(ones_col[:], 1.0)
        nc.vector.tensor_copy(vones[:, :, :, DH], ones_col[:])

        # ---- Phase A: x^T chunks (kept resident) + k/v projections ----
        # k/v projections for chunk c-1 are interleaved into chunk c's
        # transpose stream: real matmuls keep HAM warm (transpose-mode does
        # not count as PE activity) and fill the DVE-eviction stalls.
        def kvproj(c):
            kp = work_ps.tile([P, CH], F32, tag="w")
            for ks in range(NKS):
                nc.tensor.matmul(kp[:], wk_sb[:, ks, :],
                                 xt_chunks[c][:, ks, :],
                                 start=(ks == 0), stop=(ks == NKS - 1))
            nc.vector.tensor_copy(kt_sb[:, c * CH:(c + 1) * CH], kp[:])
            vp = work_ps.tile([P, CH], F32, tag="w")
            for ks in range(NKS):
                nc.tensor.matmul(vp[:], wv_sb[:, ks, :],
                                 xt_chunks[c][:, ks, :],
                                 start=(ks == 0), stop=(ks == NKS - 1))
            vt_sb = row_p.tile([P, CH], BF16, tag="vt")
            nc.vector.tensor_copy(vt_sb[:], vp[:])
            vtp = work_ps.tile([P, CH // P, P], BF16, tag="w")
            for r in range(CH // P):
                nc.tensor.transpose(vtp[:, r, :], vt_sb[:, r * P:(r + 1) * P],
                                    identity)
            for j in range(NKV):
                nc.vector.tensor_copy(
                    vones[:, j, c * (CH // P):(c + 1) * (CH // P), 0:DH],
                    vtp[:, :, j * DH:(j + 1) * DH])

        xt_chunks = []
        for c in range(NCH):
            xt_ch = xt_p.tile([P, NKS, CH], BF16, tag="xt")  # x^T[:, c*CH:+CH]
            xt_chunks.append(xt_ch)
            for r in range(CH // P):
                xrow = row_p.tile([P, D], BF16, tag="rows")
                nc.sync.dma_start(xrow[:], x_v[:, c * (CH // P) + r, :])
                # transposes batch through the (phase-A-idle) S pool so they
                # never contend with the k/v projections' work slots
                tp = s_ps.tile([P, NKS, P], BF16, tag="s")
                for dsb in range(NKS):
                    nc.tensor.transpose(
                        tp[:, dsb, :], xrow[:, dsb * P:(dsb + 1) * P], identity)
                nc.vector.tensor_copy(
                    xt_ch[:, :, r * P:(r + 1) * P], tp[:])
                # stagger weight DMAs into x-row DMA idle windows: a bulk
                # 4.5MB weight load up front stalls the xrow stream ~19us
                if c == 0 and r == 3:
                    nc.sync.dma_start(wk_sb[:], wk_v)
                    nc.sync.dma_start(wv_sb[:], wv_v)
                if c == 1 and r == 3:
                    nc.sync.dma_start(wq_sb[:], wq_v)
                if c == 2 and r == 3:
                    nc.sync.dma_start(wo_sb[:], wo_v)
                if c > 0 and r == 1:
                    kvproj(c - 1)
        kvproj(NCH - 1)

        # ---- Phase B: per T_q block: q proj (exp-hiding PE filler),
        #      attention head-pairs, Wo ----
        def qproj(tb, mb):
            xt_ch = xt_chunks[tb // 2]
            xcols = slice((tb % 2) * TQ, (tb % 2 + 1) * TQ)
            qp = work_ps.tile([P, TQ], F32, tag="w")
            for ks in range(NKS):
                nc.tensor.matmul(
                    qp[:], wq_sb[:, ks, mb * P:(mb + 1) * P],
                    xt_ch[:, ks, xcols],
                    start=(ks == 0), stop=(ks == NKS - 1))
            nc.vector.tensor_copy(qt_sb[:, mb, tb * TQ:(tb + 1) * TQ], qp[:])

        for mb in range(DQ // P):
            qproj(0, mb)
        prev = None  # (outt, tb, y_sbs) of the previous block
        for tb in range(NTQ):
            cols = slice(tb * TQ, (tb + 1) * TQ)
            outt = ot_p.tile([P, DQ // P, TQ], BF16, tag="ot")
            for p in range(TP):  # head pair (p, p+4); kv heads (0, 1)
                # both heads share one bank; start=True clears the whole
                # bank's has_written bits so ONLY the pair's first matmul
                # starts -- head B's first then overwrites-on-clear.
                pv = pv_ps.tile([P, NKV, TQ], F32, tag="pv")
                # filler for this pair, fine-grained per kg step so the PE
                # always has non-exp-dependent work while ACT streams exps:
                #   2 qproj(tb+1, mb=p) matmuls + 1 Wo(tb-1) matmul per kg
                do_q = tb + 1 < NTQ
                if do_q:
                    xt_nx = xt_chunks[(tb + 1) // 2]
                    xc_nx = slice(((tb + 1) % 2) * TQ, ((tb + 1) % 2 + 1) * TQ)
                    qp = work_ps.tile([P, TQ], F32, tag="w")
                yp = None
                for kg in range(NKI // 2):
                    sg = s_ps.tile([P, 4, TQ], F32, tag="s")
                    ex = exp_p.tile([P, 4, TQ], BF16, tag="exp")
                    for kl in range(2):
                        ki = kg * 2 + kl
                        kc = slice(ki * P, (ki + 1) * P)
                        # bank0 holds A(ki0),A(ki1); bank1 B(ki0),B(ki1)
                        # -> the concurrent (A,B) pair hits 2 banks
                        nc.tensor.matmul(
                            sg[:, kl, :], kt_sb[0:DH, kc],
                            qt_sb[0:DH, p, cols], start=True, stop=True)
                        nc.tensor.matmul(
                            sg[:, 2 + kl, :], kt_sb[DH:2 * DH, kc],
                            qt_sb[DH:2 * DH, p, cols], start=True, stop=True)
                    nc.scalar.activation(ex[:], sg[:], AF.Exp)
                    for kl in range(2):
                        ki = kg * 2 + kl
                        nc.tensor.matmul(
                            pv[0:DH + 1, 0, :], vones[:, 0, ki, :],
                            ex[:, kl, :],
                            start=(ki == 0), stop=(ki == NKI - 1),
                            skip_group_check=True)
                        nc.tensor.matmul(
                            pv[0:DH + 1, 1, :], vones[:, 1, ki, :],
                            ex[:, 2 + kl, :],
                            start=False, stop=(ki == NKI - 1),
                            skip_group_check=True)
                    # --- filler ---
                    if do_q:
                        for ks in (2 * kg, 2 * kg + 1):
                            nc.tensor.matmul(
                                qp[:], wq_sb[:, ks, p * P:(p + 1) * P],
                                xt_nx[:, ks, xc_nx],
                                start=(ks == 0), stop=(ks == NKS - 1))
                    if prev is not None:
                        po, pt, py = prev
                        g = 2 * p + kg // 4        # Wo group (mi, nb)
                        mi, nb = divmod(g, 4)
                        ks = kg % 4
                        if ks == 0:
                            yp = work_ps.tile([P, 512], F32, tag="w")
                        nc.tensor.matmul(
                            yp[:], po[:, ks, mi * P:(mi + 1) * P],
                            wo_sb[:, ks, nb * 512:(nb + 1) * 512],
                            start=(ks == 0), stop=(ks == DQ // P - 1))
                        if ks == 3:
                            nc.vector.tensor_copy(
                                py[mi][:, nb * 512:(nb + 1) * 512], yp[:])
                            if nb == 3:
                                mt = pt * (TQ // P) + mi
                                nc.sync.dma_start(y_v[:, mt, :], py[mi][:])
                if do_q:
                    nc.vector.tensor_copy(
                        qt_sb[:, p, (tb + 1) * TQ:(tb + 2) * TQ], qp[:])
                # denominator -> SBUF first: reciprocal_approx_fast's custom
                # DVE microcode mis-reads PSUM operands (verified on HW)
                den = rc_p.tile([1, NKV, TQ], F32, tag="den")
                nc.vector.tensor_copy(den[:], pv[DH:DH + 1, :, :])
                rc = rc_p.tile([1, NKV, TQ], F32, tag="rc")
                nc.vector.reciprocal_approx_fast(rc[:], den[:])
                bcst = bc_p.tile([DH, NKV, TQ], F32, tag="bc")
                nc.gpsimd.partition_broadcast(bcst[:], rc[:], channels=DH)
                nc.vector.tensor_mul(
                    outt[0:DH, p, :], pv[0:DH, 0, :], bcst[:, 0, :])
                nc.vector.tensor_mul(
                    outt[DH:2 * DH, p, :], pv[0:DH, 1, :], bcst[:, 1, :])
            y_sbs = [y_p.tile([P, D], F32, tag="y", name=f"ysb{tb}_{i}")
                     for i in range(2)]
            prev = (outt, tb, y_sbs)
        # Wo for the final block (no next block to hide it under)
        po, pt, py = prev
        for g in range(8):
            mi, nb = divmod(g, 4)
            yp = work_ps.tile([P, 512], F32, tag="w")
            for ks in range(DQ // P):
                nc.tensor.matmul(
                    yp[:], po[:, ks, mi * P:(mi + 1) * P],
                    wo_sb[:, ks, nb * 512:(nb + 1) * 512],
                    start=(ks == 0), stop=(ks == DQ // P - 1))
            nc.vector.tensor_copy(py[mi][:, nb * 512:(nb + 1) * 512], yp[:])
            if nb == 3:
                mt = pt * (TQ // P) + mi
                nc.sync.dma_start(y_v[:, mt, :], py[mi][:])

    nc.compile()
    return nc


_NC_CACHE = {}


def _get_nc():
    if "nc" not in _NC_CACHE:
        _NC_CACHE["nc"] = _build()
    return _NC_CACHE["nc"]


# q-head interleave: head h of the core -> (mb h%4, partition 64*(h//4))
_PERM = np.concatenate(
    [np.r_[b * DH:(b + 1) * DH, (b + 4) * DH:(b + 5) * DH] for b in range(4)])


def make_in_maps(x, Wq, Wk, Wv, Wo):
    x = np.asarray(x, dtype=np.float32)
    Wq = np.asarray(Wq, dtype=np.float32) * SCALE
    Wk = np.asarray(Wk, dtype=np.float32)
    Wv = np.asarray(Wv, dtype=np.float32)
    Wo = np.asarray(Wo, dtype=np.float32)
    in_maps = []
    for c in range(NCORES):
        b, g = divmod(c, TP)
        in_maps.append({
            "x": np.ascontiguousarray(x[b]).astype(BF),
            "wq": np.ascontiguousarray(
                Wq[:, g * DQ:(g + 1) * DQ][:, _PERM]).astype(BF),
            "wk": np.ascontiguousarray(Wk[:, g * DKV:(g + 1) * DKV]).astype(BF),
            "wv": np.ascontiguousarray(Wv[:, g * DKV:(g + 1) * DKV]).astype(BF),
            "wo": np.ascontiguousarray(
                Wo[g * DQ:(g + 1) * DQ, :][_PERM, :]).astype(BF),
        })
    return in_maps


def kernel(x, Wq, Wk, Wv, Wo):
    nc = _get_nc()
    in_maps = make_in_maps(x, Wq, Wk, Wv, Wo)
    res = run_bass_kernel_spmd(nc, in_maps, list(range(NCORES)))
    y = np.zeros((B, T, D), dtype=np.float32)
    for c in range(NCORES):
        b = c // TP
        y[b] += res.results[c]["y"]
    return y
